# revision 1
# baseline (speedup 1.0000x reference)
import os
import sys

sys.path.insert(0, "/opt/trn_rl_repo")
import numpy as np
from contextlib import ExitStack

import concourse.bass as bass
import concourse.bacc as bacc
import concourse.tile as tile
from concourse import mybir
from concourse.bass_utils import run_bass_kernel_spmd
from concourse.masks import make_identity

F32 = mybir.dt.float32
I16 = mybir.dt.int16
AF = mybir.ActivationFunctionType
ALU = mybir.AluOpType
AX = mybir.AxisListType

# problem constants (hardcoded per spec)
N, K, D, S, E = 16384, 32, 16, 2048, 500000
EPS = 1e-6
NCORE = 8
NCHUNK = N // 128  # 128 column-chunks of Zp
GROUP = 16  # chunks per streamed group
NGROUP = NCHUNK // GROUP  # 8
PACKW = 64  # f32 per packed row (256B, dma_gather granularity)
SROW = S // NCORE  # 256 pairwise rows per device
SRCH = SROW // 128  # 2 row-chunks of 128
EDEV = E // NCORE  # 62500 edges per device
GSZ = 4096  # edges per dma_gather op
NG_E = 16  # gathers per side
EPAD = GSZ * NG_E  # 65536
PAD_IDX = N  # index of the all-zero pad row
DIAG = float(np.sqrt(np.float32(D) * np.float32(EPS) ** 2))  # reference dist_ii

STAGE = int(os.environ.get("KERNEL_STAGE", "5"))


def _emit(nc, tc, ctx, io):
    cons = ctx.enter_context(tc.tile_pool(name="cons", bufs=1))
    zstream = ctx.enter_context(tc.tile_pool(name="zstream", bufs=1))
    stream = ctx.enter_context(tc.tile_pool(name="stream", bufs=2))
    xpool = ctx.enter_context(tc.tile_pool(name="xpool", bufs=4))
    gpool = ctx.enter_context(tc.tile_pool(name="gpool", bufs=2))
    hpool = ctx.enter_context(tc.tile_pool(name="hpool", bufs=3))
    small = ctx.enter_context(tc.tile_pool(name="small", bufs=1))
    ps_acc = ctx.enter_context(tc.tile_pool(name="ps_acc", bufs=1, space="PSUM"))
    ps_sm = ctx.enter_context(tc.tile_pool(name="ps_sm", bufs=2, space="PSUM"))
    ps4 = ctx.enter_context(tc.tile_pool(name="ps4", bufs=1, space="PSUM"))
    dram = ctx.enter_context(tc.tile_pool(name="dram", bufs=1, space="DRAM"))

    pack_dram = dram.tile([N, PACKW], F32, tag="packd")
    xpi_dram = dram.tile([N + 1, PACKW], F32, tag="xpid")
    xpj_dram = dram.tile([N + 1, PACKW], F32, tag="xpjd")

    # ---- constants / small inputs ----
    ident32 = cons.tile([32, 32], F32, tag="id32")
    make_identity(nc, ident32)
    ident128 = cons.tile([128, 128], F32, tag="id128")
    make_identity(nc, ident128)
    ones128 = cons.tile([128, 1], F32, tag="ones128")
    nc.vector.memset(ones128, 1.0)
    beta_cm = cons.tile([128, NCHUNK], F32, tag="betacm")
    nc.sync.dma_start(out=beta_cm, in_=io["betacm"][:, :])
    at_sb = cons.tile([K, D], F32, tag="at")
    nc.sync.dma_start(out=at_sb, in_=io["at"][:, :])
    sidx_sb = cons.tile([128, S // 16], I16, tag="sidx")
    nc.sync.dma_start(out=sidx_sb, in_=io["sidx"][:, :])
    sidxd_sb = cons.tile([128, SROW // 16], I16, tag="sidxd")
    nc.sync.dma_start(out=sidxd_sb, in_=io["sidxd"][:, :])
    bsf_sb = cons.tile([1, S], F32, tag="bsf")
    nc.sync.dma_start(out=bsf_sb, in_=io["bsf"][:, :])
    bsdev_sb = cons.tile([128, SRCH], F32, tag="bsdev")
    nc.sync.dma_start(out=bsdev_sb, in_=io["bsdev"][:, :])

    rs_sb = cons.tile([128, NCHUNK], F32, tag="rs")  # exp colsums (chunk-major)
    rz_sb = cons.tile([128, NCHUNK], F32, tag="rz")  # their reciprocals
    out_sb = small.tile([128, 4], F32, tag="outsb")

    def emit_out(partial=True):
        if partial:
            nc.vector.memset(out_sb, 0.0)
        nc.sync.dma_start(out=io["out"][:, :], in_=out_sb)

    # ---- stage B/C: softmax + sigmoid + pack [Z | ZTG] ----
    cs_ps = ps_acc.tile([K, 1], F32, tag="csum")  # ZTG colsum accumulator
    for g in range(NGROUP):
        zpB = zstream.tile([K, GROUP * 128], F32, tag="zpB")
        nc.sync.dma_start(out=zpB, in_=io["zp"][:, g * GROUP * 128 : (g + 1) * GROUP * 128])
        ezt = stream.tile([128, GROUP, K], F32, tag="ezt")
        for i in range(GROUP):
            c = g * GROUP + i
            tr_ps = ps_sm.tile([128, K], F32, tag="sm")
            nc.tensor.transpose(tr_ps[:, :], zpB[:, i * 128 : (i + 1) * 128], ident32[:, :])
            nc.scalar.activation(
                out=ezt[:, i, :], in_=tr_ps, func=AF.Exp, accum_out=rs_sb[:, c : c + 1]
            )
        gsl = slice(g * GROUP, (g + 1) * GROUP)
        nc.vector.reciprocal(rz_sb[:, gsl], rs_sb[:, gsl])
        gt = stream.tile([128, GROUP, K], F32, tag="gate")
        nc.sync.dma_start(
            out=gt,
            in_=io["gate"][g * GROUP * 128 : (g + 1) * GROUP * 128, :].rearrange(
                "(c p) k -> p c k", p=128
            ),
        )
        nc.scalar.activation(out=gt, in_=gt, func=AF.Sigmoid)
        pk = stream.tile([128, GROUP, PACKW], F32, tag="pack")
        # Z = exp * recip(colsum), one chunk at a time (per-partition scalar)
        for i in range(GROUP):
            c = g * GROUP + i
            nc.vector.tensor_scalar_mul(
                out=pk[:, i, 0:K], in0=ezt[:, i, :], scalar1=rz_sb[:, c : c + 1]
            )
        # ZTG = Z * sigmoid(Gate)
        nc.vector.tensor_tensor(out=pk[:, :, K : 2 * K], in0=pk[:, :, 0:K], in1=gt, op=ALU.mult)
        # accumulate ZTG colsum over n via PE
        for i in range(GROUP):
            c = g * GROUP + i
            nc.tensor.matmul(
                cs_ps[:, :],
                pk[:, i, K : 2 * K],
                ones128[:, :],
                start=(c == 0),
                stop=(c == NCHUNK - 1),
            )
        nc.sync.dma_start(
            out=pack_dram[g * GROUP * 128 : (g + 1) * GROUP * 128, :].rearrange(
                "(c p) e -> p c e", p=128
            ),
            in_=pk,
        )

    rc_sb = small.tile([K, 1], F32, tag="rc")
    nc.vector.reciprocal(rc_sb, cs_ps)
    if STAGE < 2:
        return emit_out()

    # ---- stage E: sample gather + B + AZC.T ----
    pg_sb = cons.tile([128, S // 128, PACKW], F32, tag="pg")
    nc.gpsimd.dma_gather(
        pg_sb[:, :, :],
        pack_dram[:, :],
        sidx_sb[:, :],
        num_idxs=S,
        num_idxs_reg=S,
        elem_size=PACKW,
        single_packet=False,
    )
    b_ps = ps_sm.tile([K, K], F32, tag="sm")
    for t in range(S // 128):
        nc.tensor.matmul(
            b_ps[:, :],
            pg_sb[:, t, 0:K],
            pg_sb[:, t, K : 2 * K],
            start=(t == 0),
            stop=(t == S // 128 - 1),
        )
    b_sb = small.tile([K, K], F32, tag="bsb")
    nc.vector.tensor_copy(out=b_sb, in_=b_ps)
    azct_ps = ps_sm.tile([K, D], F32, tag="sm")
    nc.tensor.matmul(azct_ps[:, :], b_sb[:, :], at_sb[:, :], start=True, stop=True)
    azct_sb = small.tile([K, D + 1], F32, tag="azct")
    nc.vector.tensor_scalar_mul(out=azct_sb[:, 0:D], in0=azct_ps, scalar1=rc_sb)
    nc.vector.memset(azct_sb[:, D : D + 1], 1.0)
    if STAGE < 3:
        return emit_out()

    # ---- stage F: X = Z.T @ AZC.T via exp-Z + per-row recip; write packs ----
    for g in range(NGROUP):
        zpF = zstream.tile([K, GROUP * 128], F32, tag="zpF")
        nc.sync.dma_start(out=zpF, in_=io["zp"][:, g * GROUP * 128 : (g + 1) * GROUP * 128])
        nc.scalar.activation(out=zpF, in_=zpF, func=AF.Exp)
        xpi = xpool.tile([128, GROUP, PACKW], F32, tag="xp")
        xpj = xpool.tile([128, GROUP, PACKW], F32, tag="xp")
        # pad lanes D+1..63 ride along in the 256B pack rows; init them
        nc.vector.memset(xpi[:, :, D + 1 : PACKW], 0.0)
        nc.vector.memset(xpj[:, :, D + 1 : PACKW], 0.0)
        for i in range(GROUP):
            x_ps = ps_sm.tile([128, D + 1], F32, tag="sm")
            nc.tensor.matmul(
                x_ps[:, :], zpF[:, i * 128 : (i + 1) * 128], azct_sb[:, :], start=True, stop=True
            )
            rzx = xpool.tile([128, 1], F32, tag="rzx")
            nc.vector.reciprocal(rzx, x_ps[:, D : D + 1])
            nc.vector.tensor_scalar(
                out=xpi[:, i, 0:D], in0=x_ps[:, 0:D], scalar1=rzx, scalar2=EPS,
                op0=ALU.mult, op1=ALU.add,
            )
            nc.vector.tensor_scalar_mul(out=xpj[:, i, 0:D], in0=x_ps[:, 0:D], scalar1=rzx)
        gsl = slice(g * GROUP, (g + 1) * GROUP)
        nc.vector.tensor_copy(out=xpi[:, :, D], in_=beta_cm[:, gsl])
        nc.vector.tensor_scalar_mul(out=xpj[:, :, D], in0=beta_cm[:, gsl], scalar1=-1.0)
        nc.sync.dma_start(
            out=xpi_dram[g * GROUP * 128 : (g + 1) * GROUP * 128, :].rearrange(
                "(c p) e -> p c e", p=128
            ),
            in_=xpi,
        )
        nc.sync.dma_start(
            out=xpj_dram[g * GROUP * 128 : (g + 1) * GROUP * 128, :].rearrange(
                "(c p) e -> p c e", p=128
            ),
            in_=xpj,
        )
    zrow = cons.tile([1, PACKW], F32, tag="zrow")
    nc.vector.memset(zrow, 0.0)
    nc.sync.dma_start(out=xpi_dram[N : N + 1, :], in_=zrow)
    nc.sync.dma_start(out=xpj_dram[N : N + 1, :], in_=zrow)
    if STAGE < 4:
        return emit_out()

    # ---- stage G: edge gathers + per-edge distance ----
    acc_sb = cons.tile([128, NG_E * (GSZ // 128)], F32, tag="acc")
    for gi in range(NG_E):
        isl = slice(gi * (GSZ // 16), (gi + 1) * (GSZ // 16))
        ei_sb = gpool.tile([128, GSZ // 16], I16, tag="ei")
        nc.sync.dma_start(out=ei_sb, in_=io["ei"][:, isl])
        xi_t = gpool.tile([128, GSZ // 128, PACKW], F32, tag="xi")
        nc.gpsimd.dma_gather(
            xi_t[:, :, :], xpi_dram[:, :], ei_sb[:, :],
            num_idxs=GSZ, num_idxs_reg=GSZ, elem_size=PACKW, single_packet=False,
        )
        ej_sb = gpool.tile([128, GSZ // 16], I16, tag="ej")
        nc.sync.dma_start(out=ej_sb, in_=io["ej"][:, isl])
        xj_t = gpool.tile([128, GSZ // 128, PACKW], F32, tag="xj")
        nc.gpsimd.dma_gather(
            xj_t[:, :, :], xpj_dram[:, :], ej_sb[:, :],
            num_idxs=GSZ, num_idxs_reg=GSZ, elem_size=PACKW, single_packet=False,
        )
        dt_t = gpool.tile([128, GSZ // 128, D + 1], F32, tag="dt")
        nc.vector.tensor_tensor(
            out=dt_t, in0=xi_t[:, :, 0 : D + 1], in1=xj_t[:, :, 0 : D + 1], op=ALU.subtract
        )
        sqe_t = gpool.tile([128, GSZ // 128, D], F32, tag="sqe")
        nc.vector.tensor_tensor(
            out=sqe_t, in0=dt_t[:, :, 0:D], in1=dt_t[:, :, 0:D], op=ALU.mult
        )
        d2_t = gpool.tile([128, GSZ // 128], F32, tag="d2")
        nc.vector.reduce_sum(d2_t, sqe_t, axis=AX.X)
        ed_t = gpool.tile([128, GSZ // 128], F32, tag="ed")
        nc.scalar.sqrt(ed_t, d2_t)
        csl = slice(gi * (GSZ // 128), (gi + 1) * (GSZ // 128))
        nc.vector.tensor_tensor(
            out=acc_sb[:, csl], in0=dt_t[:, :, D], in1=ed_t, op=ALU.subtract
        )

    nc.vector.reduce_sum(out_sb[:, 0:1], acc_sb, axis=AX.X)
    if STAGE < 5:
        nc.vector.memset(out_sb[:, 1:4], 0.0)
        return emit_out(partial=False)

    # ---- stage H: S x S pairwise block (device rows only) ----
    zsk_sb = cons.tile([K, S], F32, tag="zsk")
    for t in range(S // 128):
        zs_ps = ps_sm.tile([K, 128], F32, tag="sm")
        nc.tensor.transpose(zs_ps[:, :], pg_sb[:, t, 0:K], ident128[:, :])
        nc.scalar.copy(zsk_sb[:, t * 128 : (t + 1) * 128], zs_ps)
    pgd_sb = cons.tile([128, SRCH, PACKW], F32, tag="pgd")
    nc.gpsimd.dma_gather(
        pgd_sb[:, :, :], pack_dram[:, :], sidxd_sb[:, :],
        num_idxs=SROW, num_idxs_reg=SROW, elem_size=PACKW,
        single_packet=False,
    )
    zskd_sb = cons.tile([K, SROW], F32, tag="zskd")
    for t in range(SRCH):
        zs_ps = ps_sm.tile([K, 128], F32, tag="sm")
        nc.tensor.transpose(zs_ps[:, :], pgd_sb[:, t, 0:K], ident128[:, :])
        nc.scalar.copy(zskd_sb[:, t * 128 : (t + 1) * 128], zs_ps)

    # azT full [D, S] = AZC @ Zs, plus its squares for the column term
    azf_ps = ps4.tile([128, S], F32, tag="wide")
    for f in range(S // 512):
        nc.tensor.matmul(
            azf_ps[0:D, f * 512 : (f + 1) * 512],
            azct_sb[:, 0:D],
            zsk_sb[:, f * 512 : (f + 1) * 512],
            start=True,
            stop=True,
        )
    sq_sb = small.tile([D, S], F32, tag="sqf")
    nc.scalar.square(sq_sb, azf_ps[0:D, :])
    ga_sb = small.tile([D + 1, S], F32, tag="ga")
    nc.scalar.copy(ga_sb[0:D, :], azf_ps[0:D, :])
    cwa_sb = small.tile([D, 1], F32, tag="cwa")
    nc.vector.memset(cwa_sb, EPS)
    cwb_sb = small.tile([D, 1], F32, tag="cwb")
    nc.vector.memset(cwb_sb, -0.5)
    ct_ps = ps4.tile([128, S], F32, tag="wide")
    for f in range(S // 512):
        fsl = slice(f * 512, (f + 1) * 512)
        nc.tensor.matmul(ct_ps[0:1, fsl], cwa_sb[:, :], ga_sb[0:D, fsl], start=True, stop=False)
        nc.tensor.matmul(ct_ps[0:1, fsl], cwb_sb[:, :], sq_sb[:, fsl], start=False, stop=True)
    ctrow_sb = small.tile([1, S], F32, tag="ctrow")
    nc.scalar.copy(ctrow_sb, ct_ps[0:1, :])
    # row 16 of the augmented rhs, via DMA (compute engines cannot start at partition 16)
    nc.sync.dma_start(out=ga_sb[D : D + 1, :], in_=ctrow_sb)

    # device-row azT (augmented with ones row) and R terms
    azd_ps = ps_sm.tile([D, SROW], F32, tag="sm")
    nc.tensor.matmul(azd_ps[:, :], azct_sb[:, 0:D], zskd_sb[:, :], start=True, stop=True)
    azd_aug = small.tile([D + 1, SROW], F32, tag="azdaug")
    nc.scalar.copy(azd_aug[0:D, :], azd_ps)
    ones_row = small.tile([1, SROW], F32, tag="onesrow")
    nc.vector.memset(ones_row, 1.0)
    nc.sync.dma_start(out=azd_aug[D : D + 1, :], in_=ones_row)
    r_sb = small.tile([128, SRCH], F32, tag="rsb")
    sa_sb = small.tile([128, SRCH], F32, tag="sasb")
    rn_sb = small.tile([128, SRCH], F32, tag="rnsb")
    for t in range(SRCH):
        azdn_ps = ps_sm.tile([128, D], F32, tag="sm")
        nc.tensor.matmul(
            azdn_ps[:, :],
            zskd_sb[:, t * 128 : (t + 1) * 128],
            azct_sb[:, 0:D],
            start=True,
            stop=True,
        )
        nc.vector.reduce_sum(sa_sb[:, t : t + 1], azdn_ps, axis=AX.X)
        sqn_t = small.tile([128, D], F32, tag="sqn")
        nc.scalar.square(sqn_t, azdn_ps)
        nc.vector.reduce_sum(rn_sb[:, t : t + 1], sqn_t, axis=AX.X)
    nc.vector.tensor_scalar(
        out=r_sb, in0=sa_sb, scalar1=2.0 * EPS, scalar2=float(D) * EPS * EPS,
        op0=ALU.mult, op1=ALU.add,
    )
    nc.vector.tensor_tensor(out=r_sb, in0=r_sb, in1=rn_sb, op=ALU.add)

    # w_rep = exp(bs) broadcast to all partitions
    nc.scalar.activation(out=bsf_sb, in_=bsf_sb, func=AF.Exp)
    ones1 = small.tile([1, 128], F32, tag="ones1")
    nc.vector.memset(ones1, 1.0)
    wr_ps = ps4.tile([128, S], F32, tag="wide")
    for f in range(S // 512):
        fsl = slice(f * 512, (f + 1) * 512)
        nc.tensor.matmul(wr_ps[:, fsl], ones1[:, :], bsf_sb[:, fsl], start=True, stop=True)
    wr_sb = small.tile([128, S], F32, tag="wr")
    nc.vector.tensor_copy(out=wr_sb, in_=wr_ps)

    rp_sb = small.tile([128, SRCH], F32, tag="rp")
    for r in range(SRCH):
        gr_ps = ps4.tile([128, S], F32, tag="wide")
        for f in range(S // 512):
            fsl = slice(f * 512, (f + 1) * 512)
            nc.tensor.matmul(
                gr_ps[:, fsl],
                azd_aug[:, r * 128 : (r + 1) * 128],
                ga_sb[:, fsl],
                start=True,
                stop=True,
            )
        d2t = hpool.tile([128, S], F32, tag="h2k")
        nc.vector.tensor_scalar(
            out=d2t, in0=gr_ps, scalar1=-2.0, scalar2=r_sb[:, r : r + 1],
            op0=ALU.mult, op1=ALU.add,
        )
        nc.vector.tensor_scalar_max(out=d2t, in0=d2t, scalar1=0.0)
        dst = hpool.tile([128, S], F32, tag="h2k")
        nc.scalar.sqrt(dst, d2t)
        ex_t = hpool.tile([128, S], F32, tag="h2k")
        nc.scalar.activation(
            out=ex_t, in_=dst, func=AF.Exp, bias=bsdev_sb[:, r : r + 1], scale=-1.0
        )
        junk = hpool.tile([128, S], F32, tag="h2k")
        nc.vector.tensor_tensor(out=junk, in0=ex_t, in1=wr_sb, op=ALU.mult)
        nc.vector.reduce_sum(rp_sb[:, r : r + 1], junk, axis=AX.X)
    dg_sb = small.tile([128, SRCH], F32, tag="dg")
    dgb_sb = small.tile([128, 1], F32, tag="dgb")
    nc.vector.memset(dgb_sb, -DIAG)
    nc.scalar.activation(out=dg_sb, in_=bsdev_sb, func=AF.Exp, bias=dgb_sb, scale=2.0)
    nc.vector.tensor_tensor(out=out_sb[:, 1 : 1 + SRCH], in0=rp_sb, in1=dg_sb, op=ALU.subtract)
    nc.vector.memset(out_sb[:, 3:4], 0.0)
    emit_out(partial=False)


def _build_program():
    nc = bacc.Bacc(None, target_bir_lowering=False, debug=False)

    io = {
        "zp": nc.declare_dram_parameter("zp", [K, N], F32, isOutput=False),
        "gate": nc.declare_dram_parameter("gate", [N, K], F32, isOutput=False),
        "betacm": nc.declare_dram_parameter("betacm", [128, NCHUNK], F32, isOutput=False),
        "at": nc.declare_dram_parameter("at", [K, D], F32, isOutput=False),
        "sidx": nc.declare_dram_parameter("sidx", [128, S // 16], I16, isOutput=False),
        "sidxd": nc.declare_dram_parameter("sidxd", [128, SROW // 16], I16, isOutput=False),
        "bsf": nc.declare_dram_parameter("bsf", [1, S], F32, isOutput=False),
        "bsdev": nc.declare_dram_parameter("bsdev", [128, SRCH], F32, isOutput=False),
        "ei": nc.declare_dram_parameter("ei", [128, EPAD // 16], I16, isOutput=False),
        "ej": nc.declare_dram_parameter("ej", [128, EPAD // 16], I16, isOutput=False),
        "out": nc.declare_dram_parameter("out", [128, 4], F32, isOutput=True),
    }

    with tile.TileContext(nc) as tc, ExitStack() as ctx:
        _emit(nc, tc, ctx, io)

    nc.compile()
    return nc


_NC_CACHE = None
LAST_RESULTS = None


def _get_program():
    global _NC_CACHE
    if _NC_CACHE is None:
        _NC_CACHE = _build_program()
    return _NC_CACHE


def _wrap_idx(idx, pad_to):
    """Wrap indices into the [128, pad_to//16] int16 layout dma_gather expects."""
    n = idx.shape[0]
    padded = np.full(pad_to, PAD_IDX, dtype=np.int64)
    padded[:n] = idx
    w = padded.reshape(pad_to // 16, 16).T.astype(np.int16)  # [16, L]
    return np.ascontiguousarray(np.tile(w, (8, 1)))  # [128, L]


def kernel(beta, A, Zp, Gate, sample_idx, sparse_i, sparse_j):
    beta = np.ascontiguousarray(np.asarray(beta, dtype=np.float32))
    A = np.ascontiguousarray(np.asarray(A, dtype=np.float32))
    Zp = np.ascontiguousarray(np.asarray(Zp, dtype=np.float32))
    Gate = np.ascontiguousarray(np.asarray(Gate, dtype=np.float32))
    si = np.asarray(sample_idx).astype(np.int64)
    ei = np.asarray(sparse_i).astype(np.int64)
    ej = np.asarray(sparse_j).astype(np.int64)
    # Sort edges by (i, j): the edge sum is order-invariant, and sorted indices
    # turn random 256B row gathers into mostly-ascending accesses (HBM
    # row-buffer locality), each device touching ~1/8 of the table on the i side.
    order = np.lexsort((ej, ei))
    ei = ei[order]
    ej = ej[order]

    nc = _get_program()

    beta_cm = np.ascontiguousarray(beta.reshape(NCHUNK, 128).T)
    at = np.ascontiguousarray(A.T)
    bs = beta[si]
    bsf = np.ascontiguousarray(bs.reshape(1, S))
    sidx_w = _wrap_idx(si, S)

    in_maps = []
    for d in range(NCORE):
        rows = si[d * SROW : (d + 1) * SROW]
        bsdev = np.ascontiguousarray(bs[d * SROW : (d + 1) * SROW].reshape(SRCH, 128).T)
        in_maps.append(
            dict(
                zp=Zp,
                gate=Gate,
                betacm=beta_cm,
                at=at,
                sidx=sidx_w,
                sidxd=_wrap_idx(rows, SROW),
                bsf=bsf,
                bsdev=bsdev,
                ei=_wrap_idx(ei[d * EDEV : (d + 1) * EDEV], EPAD),
                ej=_wrap_idx(ej[d * EDEV : (d + 1) * EDEV], EPAD),
            )
        )

    trace = os.environ.get("BASS_KERNEL_TRACE") == "1"
    res = run_bass_kernel_spmd(nc, in_maps, list(range(NCORE)), trace=trace)
    global LAST_RESULTS
    LAST_RESULTS = res

    z2 = 0.0
    z1 = 0.0
    for d in range(NCORE):
        o = np.asarray(res.results[d]["out"], dtype=np.float64)
        z2 += o[:, 0].sum()
        z1 += o[:, 1 : 1 + SRCH].sum()
    e = np.float64(np.exp(np.float32(1.0)))
    return np.float32(z2 - 0.5 * e * e * z1)



# revision 6
# speedup vs baseline: 2.2051x; 2.2051x over previous
import os
import sys

sys.path.insert(0, "/opt/trn_rl_repo")
import numpy as np
from contextlib import ExitStack

import concourse.bass as bass
import concourse.bacc as bacc
import concourse.tile as tile
from concourse import mybir
from concourse.bass_utils import run_bass_kernel_spmd
from concourse.masks import make_identity

F32 = mybir.dt.float32
I16 = mybir.dt.int16
AF = mybir.ActivationFunctionType
ALU = mybir.AluOpType
AX = mybir.AxisListType

# problem constants (hardcoded per spec)
N, K, D, S, E = 16384, 32, 16, 2048, 500000
EPS = 1e-6
NCORE = 8
NTILE = 32  # [128,128] restacked tiles of Zp (4 node-chunks each)
SROW = S // NCORE  # 256 pairwise rows per device
SRCH = SROW // 128  # 2 row-chunks of 128
EDEV = E // NCORE  # 62500 edges per device
GSZ = 4096  # edges per dma_gather op
NG_E = 16  # gathers per side
EPAD = GSZ * NG_E  # 65536
PAD_IDX = N  # index of the all-zero pad row
DIAG = float(np.sqrt(np.float32(D) * np.float32(EPS) ** 2))  # reference dist_ii


def _emit(nc, tc, ctx, io):
    cons = ctx.enter_context(tc.tile_pool(name="cons", bufs=1))
    zstream = ctx.enter_context(tc.tile_pool(name="zstream", bufs=2))
    stream = ctx.enter_context(tc.tile_pool(name="stream", bufs=3))
    gpool = ctx.enter_context(tc.tile_pool(name="gpool", bufs=4))
    dpool = ctx.enter_context(tc.tile_pool(name="dpool", bufs=4))
    hpool = ctx.enter_context(tc.tile_pool(name="hpool", bufs=3))
    small = ctx.enter_context(tc.tile_pool(name="small", bufs=1))
    ps_sm = ctx.enter_context(tc.tile_pool(name="ps_sm", bufs=2, space="PSUM"))
    ps4 = ctx.enter_context(tc.tile_pool(name="ps4", bufs=1, space="PSUM"))
    dram = ctx.enter_context(tc.tile_pool(name="dram", bufs=1, space="DRAM"))

    pack_dram = dram.tile([N, 64], F32, tag="packd")  # [ez | ez*sigmoid(gate)]
    xt_dram = dram.tile([N + 1, 64], F32, tag="xtd")  # x rows (cols 0:D valid)

    # ---- constants / small inputs (loaded up front) ----
    ident128 = cons.tile([128, 128], F32, tag="id128")
    make_identity(nc, ident128)
    ident1 = cons.tile([1, 1], F32, tag="id1")
    nc.vector.memset(ident1, 1.0)
    ones128 = cons.tile([128, 1], F32, tag="ones128")
    nc.vector.memset(ones128, 1.0)
    betacm = cons.tile([128, 128], F32, tag="betacm")
    nc.sync.dma_start(out=betacm, in_=io["betacm"][:, :])
    degcm = cons.tile([128, 128], F32, tag="degcm")
    nc.sync.dma_start(out=degcm, in_=io["degcm"][:, :])
    at_sb = cons.tile([K, D], F32, tag="at")
    nc.sync.dma_start(out=at_sb, in_=io["at"][:, :])
    sidx_sb = cons.tile([128, S // 16], I16, tag="sidx")
    nc.sync.dma_start(out=sidx_sb, in_=io["sidx"][:, :])
    sidxd_sb = cons.tile([128, SROW // 16], I16, tag="sidxd")
    nc.sync.dma_start(out=sidxd_sb, in_=io["sidxd"][:, :])
    bsf_sb = cons.tile([1, S], F32, tag="bsf")
    nc.sync.dma_start(out=bsf_sb, in_=io["bsf"][:, :])
    bsdev_sb = cons.tile([128, SRCH], F32, tag="bsdev")
    nc.sync.dma_start(out=bsdev_sb, in_=io["bsdev"][:, :])
    ei_sb = cons.tile([128, EPAD // 16], I16, tag="ei")
    nc.sync.dma_start(out=ei_sb, in_=io["ei"][:, :])
    ej_sb = cons.tile([128, EPAD // 16], I16, tag="ej")
    nc.sync.dma_start(out=ej_sb, in_=io["ej"][:, :])

    ez4 = cons.tile([128, NTILE * 128], F32, tag="ez4")  # exp(Zp) restacked, kept
    out_sb = small.tile([128, 4], F32, tag="outsb")

    # ---- stage B: exp, pack [ez | ez*sig(gate)], ZTG colsum accumulator ----
    for c in range(4):
        zp4 = zstream.tile([128, NTILE * 32], F32, tag="zp4")
        sl = slice(c * NTILE * 32, (c + 1) * NTILE * 32)
        nc.sync.dma_start(out=zp4, in_=io["zp4"][:, sl])
        nc.scalar.activation(out=ez4[:, sl], in_=zp4, func=AF.Exp)
    acc4 = cons.tile([128, 4, 32], F32, tag="acc4")
    nc.vector.memset(acc4, 0.0)
    for t in range(NTILE):
        tr_ps = ps_sm.tile([128, 128], F32, tag="sm")
        nc.tensor.transpose(tr_ps[:, :], ez4[:, t * 128 : (t + 1) * 128], ident128[:, :])
        gt = stream.tile([128, 4, 32], F32, tag="gate")
        nc.sync.dma_start(
            out=gt,
            in_=io["gate"][t * 512 : (t + 1) * 512, :].rearrange("(b p) k -> p b k", p=128),
        )
        nc.scalar.activation(out=gt, in_=gt, func=AF.Sigmoid)
        pk = stream.tile([128, 4, 64], F32, tag="pack")
        nc.vector.tensor_copy(
            out=pk[:, :, 0:32], in_=tr_ps[:, :].rearrange("p (b k) -> p b k", b=4)
        )
        nc.vector.tensor_tensor(
            out=pk[:, :, 32:64], in0=pk[:, :, 0:32], in1=gt, op=ALU.mult
        )
        nc.vector.tensor_tensor(out=acc4, in0=acc4, in1=pk[:, :, 32:64], op=ALU.add)
        nc.sync.dma_start(
            out=pack_dram[t * 512 : (t + 1) * 512, :].rearrange("(b p) e -> p b e", p=128),
            in_=pk,
        )

    # ztg colsum c[K] -> rc = 1/c as [K,1]
    cs_ps = ps_sm.tile([128, 1], F32, tag="sm")
    nc.tensor.matmul(
        cs_ps[:, :],
        acc4[:, :, :].rearrange("p b k -> p (b k)"),
        ones128[:, :],
        start=True,
        stop=True,
    )
    cs_sb = small.tile([128, 1], F32, tag="cssb")
    nc.vector.tensor_copy(out=cs_sb, in_=cs_ps)
    csr_ps = ps_sm.tile([1, 128], F32, tag="sm")
    nc.tensor.transpose(csr_ps[:, :], cs_sb[:, :], ident128[:, :])
    csr_sb = small.tile([1, 128], F32, tag="csrsb")
    nc.vector.tensor_copy(out=csr_sb, in_=csr_ps)
    c_row = small.tile([1, 32], F32, tag="crow")
    nc.vector.reduce_sum(c_row, csr_sb[0:1, :].rearrange("p (b k) -> p k b", b=4), axis=AX.X)
    rc_row = small.tile([1, 32], F32, tag="rcrow")
    nc.vector.reciprocal(rc_row, c_row)
    rc_ps = ps_sm.tile([32, 1], F32, tag="sm")
    nc.tensor.transpose(rc_ps[:, :], rc_row[:, :], ident1[:, :])
    rc_sb = small.tile([32, 1], F32, tag="rcsb")
    nc.vector.tensor_copy(out=rc_sb, in_=rc_ps)

    # ---- stage E: sample gather (2 queues) + normalize + B + azct ----
    pg_sb = cons.tile([128, S // 128, 64], F32, tag="pg")
    for h in range(2):
        nc.gpsimd.dma_gather(
            pg_sb[:, h * 8 : (h + 1) * 8, :],
            pack_dram[:, :],
            sidx_sb[:, h * (S // 32) : (h + 1) * (S // 32)],
            num_idxs=S // 2,
            num_idxs_reg=S // 2,
            elem_size=64,
            single_packet=False,
            queue_num=h,
        )
    pgs = small.tile([128, S // 128], F32, tag="pgs")
    nc.vector.reduce_sum(pgs, pg_sb[:, :, 0:32], axis=AX.X)
    pgr = small.tile([128, S // 128], F32, tag="pgr")
    nc.vector.reciprocal(pgr, pgs)
    for t in range(S // 128):
        nc.vector.tensor_scalar_mul(
            out=pg_sb[:, t, 0:32], in0=pg_sb[:, t, 0:32], scalar1=pgr[:, t : t + 1]
        )
        nc.vector.tensor_scalar_mul(
            out=pg_sb[:, t, 32:64], in0=pg_sb[:, t, 32:64], scalar1=pgr[:, t : t + 1]
        )
    b_ps = ps_sm.tile([K, K], F32, tag="sm")
    for t in range(S // 128):
        nc.tensor.matmul(
            b_ps[:, :],
            pg_sb[:, t, 0:K],
            pg_sb[:, t, K : 2 * K],
            start=(t == 0),
            stop=(t == S // 128 - 1),
        )
    b_sb = small.tile([K, K], F32, tag="bsb")
    nc.vector.tensor_copy(out=b_sb, in_=b_ps)
    azct_ps = ps_sm.tile([K, D], F32, tag="sm")
    nc.tensor.matmul(azct_ps[:, :], b_sb[:, :], at_sb[:, :], start=True, stop=True)
    azct_sb = small.tile([K, D + 1], F32, tag="azct")
    nc.vector.tensor_scalar_mul(out=azct_sb[:, 0:D], in0=azct_ps, scalar1=rc_sb)
    nc.vector.memset(azct_sb[:, D : D + 1], 1.0)

    # block-diagonal [128,68] weights: rows (b,k); col b*17+e = azct[k,e] for
    # e<16, col b*17+16 = 1.0 (softmax denominator s)
    azbd = cons.tile([128, 4, D + 1], F32, tag="azbd")
    nc.vector.memset(azbd, 0.0)
    for b in range(4):
        nc.vector.tensor_copy(
            out=azbd[b * 32 : (b + 1) * 32, b, 0:D], in_=azct_sb[:, 0:D]
        )
        nc.vector.memset(azbd[b * 32 : (b + 1) * 32, b, D : D + 1], 1.0)

    # ---- stage F: X table (normalized), written as [N,64] rows (cols 0:D) ----
    zrow = small.tile([1, 64], F32, tag="zrow")
    nc.vector.memset(zrow, 0.0)
    nc.sync.dma_start(out=xt_dram[N : N + 1, :], in_=zrow)
    for t in range(NTILE):
        xf_ps = ps_sm.tile([128, 4, D + 1], F32, tag="sm")
        nc.tensor.matmul(
            xf_ps[:, :, :].rearrange("p b e -> p (b e)"),
            ez4[:, t * 128 : (t + 1) * 128],
            azbd[:, :, :].rearrange("p b e -> p (b e)"),
            start=True,
            stop=True,
        )
        rr4 = dpool.tile([128, 4], F32, tag="rr")
        nc.vector.reciprocal(rr4, xf_ps[:, :, D])
        xr = dpool.tile([128, 4, D], F32, tag="xn")
        for b in range(4):
            nc.vector.tensor_scalar_mul(
                out=xr[:, b, :], in0=xf_ps[:, b, 0:D], scalar1=rr4[:, b : b + 1]
            )
        nc.sync.dma_start(
            out=xt_dram[t * 512 : (t + 1) * 512, 0:D].rearrange("(b p) d -> p b d", p=128),
            in_=xr,
        )

    # ---- stage H: S x S pairwise block (device rows only) ----
    zsk_sb = cons.tile([K, S], F32, tag="zsk")
    for t in range(S // 128):
        zs_ps = ps_sm.tile([K, 128], F32, tag="sm")
        nc.tensor.transpose(zs_ps[:, :], pg_sb[:, t, 0:K], ident128[:, :])
        nc.scalar.copy(zsk_sb[:, t * 128 : (t + 1) * 128], zs_ps)
    pgd_sb = cons.tile([128, SRCH, 64], F32, tag="pgd")
    nc.gpsimd.dma_gather(
        pgd_sb[:, :, :],
        pack_dram[:, :],
        sidxd_sb[:, :],
        num_idxs=SROW,
        num_idxs_reg=SROW,
        elem_size=64,
        single_packet=False,
        queue_num=2,
    )
    pds = small.tile([128, SRCH], F32, tag="pds")
    nc.vector.reduce_sum(pds, pgd_sb[:, :, 0:32], axis=AX.X)
    pdr = small.tile([128, SRCH], F32, tag="pdr")
    nc.vector.reciprocal(pdr, pds)
    for t in range(SRCH):
        nc.vector.tensor_scalar_mul(
            out=pgd_sb[:, t, 0:32], in0=pgd_sb[:, t, 0:32], scalar1=pdr[:, t : t + 1]
        )
    zskd_sb = cons.tile([K, SROW], F32, tag="zskd")
    for t in range(SRCH):
        zs_ps = ps_sm.tile([K, 128], F32, tag="sm")
        nc.tensor.transpose(zs_ps[:, :], pgd_sb[:, t, 0:K], ident128[:, :])
        nc.scalar.copy(zskd_sb[:, t * 128 : (t + 1) * 128], zs_ps)

    # azT full [D, S] = AZC @ Zs, plus its squares for the column term
    azf_ps = ps4.tile([128, S], F32, tag="wide")
    for f in range(S // 512):
        nc.tensor.matmul(
            azf_ps[0:D, f * 512 : (f + 1) * 512],
            azct_sb[:, 0:D],
            zsk_sb[:, f * 512 : (f + 1) * 512],
            start=True,
            stop=True,
        )
    sq_sb = small.tile([D, S], F32, tag="sqf")
    nc.scalar.square(sq_sb, azf_ps[0:D, :])
    ga_sb = small.tile([D + 1, S], F32, tag="ga")
    nc.scalar.copy(ga_sb[0:D, :], azf_ps[0:D, :])
    cwa_sb = small.tile([D, 1], F32, tag="cwa")
    nc.vector.memset(cwa_sb, EPS)
    cwb_sb = small.tile([D, 1], F32, tag="cwb")
    nc.vector.memset(cwb_sb, -0.5)
    ct_ps = ps4.tile([128, S], F32, tag="wide")
    for f in range(S // 512):
        fsl = slice(f * 512, (f + 1) * 512)
        nc.tensor.matmul(ct_ps[0:1, fsl], cwa_sb[:, :], ga_sb[0:D, fsl], start=True, stop=False)
        nc.tensor.matmul(ct_ps[0:1, fsl], cwb_sb[:, :], sq_sb[:, fsl], start=False, stop=True)
    ctrow_sb = small.tile([1, S], F32, tag="ctrow")
    nc.scalar.copy(ctrow_sb, ct_ps[0:1, :])
    # row 16 of the augmented rhs, via DMA (compute engines cannot start at partition 16)
    nc.sync.dma_start(out=ga_sb[D : D + 1, :], in_=ctrow_sb)

    # device-row azT (augmented with ones row) and R terms
    azd_ps = ps_sm.tile([D, SROW], F32, tag="sm")
    nc.tensor.matmul(azd_ps[:, :], azct_sb[:, 0:D], zskd_sb[:, :], start=True, stop=True)
    azd_aug = small.tile([D + 1, SROW], F32, tag="azdaug")
    nc.scalar.copy(azd_aug[0:D, :], azd_ps)
    ones_row = small.tile([1, SROW], F32, tag="onesrow")
    nc.vector.memset(ones_row, 1.0)
    nc.sync.dma_start(out=azd_aug[D : D + 1, :], in_=ones_row)
    r_sb = small.tile([128, SRCH], F32, tag="rsb")
    sa_sb = small.tile([128, SRCH], F32, tag="sasb")
    rn_sb = small.tile([128, SRCH], F32, tag="rnsb")
    for t in range(SRCH):
        azdn_ps = ps_sm.tile([128, D], F32, tag="sm")
        nc.tensor.matmul(
            azdn_ps[:, :],
            zskd_sb[:, t * 128 : (t + 1) * 128],
            azct_sb[:, 0:D],
            start=True,
            stop=True,
        )
        nc.vector.reduce_sum(sa_sb[:, t : t + 1], azdn_ps, axis=AX.X)
        sqn_t = small.tile([128, D], F32, tag="sqn")
        nc.scalar.square(sqn_t, azdn_ps)
        nc.vector.reduce_sum(rn_sb[:, t : t + 1], sqn_t, axis=AX.X)
    nc.vector.tensor_scalar(
        out=r_sb, in0=sa_sb, scalar1=2.0 * EPS, scalar2=float(D) * EPS * EPS,
        op0=ALU.mult, op1=ALU.add,
    )
    nc.vector.tensor_tensor(out=r_sb, in0=r_sb, in1=rn_sb, op=ALU.add)

    # w_rep = exp(bs) broadcast to all partitions
    nc.scalar.activation(out=bsf_sb, in_=bsf_sb, func=AF.Exp)
    ones1 = small.tile([1, 128], F32, tag="ones1")
    nc.vector.memset(ones1, 1.0)
    wr_ps = ps4.tile([128, S], F32, tag="wide")
    for f in range(S // 512):
        fsl = slice(f * 512, (f + 1) * 512)
        nc.tensor.matmul(wr_ps[:, fsl], ones1[:, :], bsf_sb[:, fsl], start=True, stop=True)
    wr_sb = small.tile([128, S], F32, tag="wr")
    nc.vector.tensor_copy(out=wr_sb, in_=wr_ps)

    rp_sb = small.tile([128, SRCH], F32, tag="rp")
    for r in range(SRCH):
        gr_ps = ps4.tile([128, S], F32, tag="wide")
        for f in range(S // 512):
            fsl = slice(f * 512, (f + 1) * 512)
            nc.tensor.matmul(
                gr_ps[:, fsl],
                azd_aug[:, r * 128 : (r + 1) * 128],
                ga_sb[:, fsl],
                start=True,
                stop=True,
            )
        d2t = hpool.tile([128, S], F32, tag="h2k")
        nc.vector.tensor_scalar(
            out=d2t, in0=gr_ps, scalar1=-2.0, scalar2=r_sb[:, r : r + 1],
            op0=ALU.mult, op1=ALU.add,
        )
        nc.vector.tensor_scalar_max(out=d2t, in0=d2t, scalar1=0.0)
        dst = hpool.tile([128, S], F32, tag="h2k")
        nc.scalar.sqrt(dst, d2t)
        ex_t = hpool.tile([128, S], F32, tag="h2k")
        nc.scalar.activation(
            out=ex_t, in_=dst, func=AF.Exp, bias=bsdev_sb[:, r : r + 1], scale=-1.0
        )
        junk = hpool.tile([128, S], F32, tag="h2k")
        nc.vector.tensor_tensor(out=junk, in0=ex_t, in1=wr_sb, op=ALU.mult)
        nc.vector.reduce_sum(rp_sb[:, r : r + 1], junk, axis=AX.X)
    dg_sb = small.tile([128, SRCH], F32, tag="dg")
    dgb_sb = small.tile([128, 1], F32, tag="dgb")
    nc.vector.memset(dgb_sb, -DIAG)
    nc.scalar.activation(out=dg_sb, in_=bsdev_sb, func=AF.Exp, bias=dgb_sb, scale=2.0)
    nc.vector.tensor_tensor(out=out_sb[:, 1 : 1 + SRCH], in0=rp_sb, in1=dg_sb, op=ALU.subtract)
    nc.vector.memset(out_sb[:, 3:4], 0.0)

    # ---- stage G: edge gathers (4 queues) + per-edge distance ----
    acc_sb = cons.tile([128, NG_E * (GSZ // 128)], F32, tag="acc")
    for gi in range(NG_E):
        isl = slice(gi * (GSZ // 16), (gi + 1) * (GSZ // 16))
        xi_t = gpool.tile([128, GSZ // 128, 64], F32, tag="xi")
        nc.gpsimd.dma_gather(
            xi_t[:, :, :], xt_dram[:, :], ei_sb[:, isl],
            num_idxs=GSZ, num_idxs_reg=GSZ, elem_size=64,
            single_packet=False, queue_num=(2 * gi) % 4,
        )
        xj_t = gpool.tile([128, GSZ // 128, 64], F32, tag="xj")
        nc.gpsimd.dma_gather(
            xj_t[:, :, :], xt_dram[:, :], ej_sb[:, isl],
            num_idxs=GSZ, num_idxs_reg=GSZ, elem_size=64,
            single_packet=False, queue_num=(2 * gi + 1) % 4,
        )
        dt_t = dpool.tile([128, GSZ // 128, D], F32, tag="dt")
        nc.vector.tensor_tensor(
            out=dt_t, in0=xi_t[:, :, 0:D], in1=xj_t[:, :, 0:D], op=ALU.subtract
        )
        nc.vector.tensor_scalar_add(out=dt_t, in0=dt_t, scalar1=EPS)
        nc.vector.tensor_tensor(out=dt_t, in0=dt_t, in1=dt_t, op=ALU.mult)
        d2_t = dpool.tile([128, GSZ // 128], F32, tag="d2")
        nc.vector.reduce_sum(d2_t, dt_t, axis=AX.X)
        csl = slice(gi * (GSZ // 128), (gi + 1) * (GSZ // 128))
        nc.scalar.sqrt(acc_sb[:, csl], d2_t)

    # z2 partial: beta*deg (device 0 only via input) - sum of edge distances
    bd_t = small.tile([128, 128], F32, tag="bd")
    nc.vector.tensor_tensor(out=bd_t, in0=betacm, in1=degcm, op=ALU.mult)
    bsum = small.tile([128, 1], F32, tag="bsum")
    nc.vector.reduce_sum(bsum, bd_t, axis=AX.X)
    esum = small.tile([128, 1], F32, tag="esum")
    nc.vector.reduce_sum(esum, acc_sb, axis=AX.X)
    nc.vector.tensor_tensor(out=out_sb[:, 0:1], in0=bsum, in1=esum, op=ALU.subtract)
    nc.sync.dma_start(out=io["out"][:, :], in_=out_sb)


def _build_program():
    nc = bacc.Bacc(None, target_bir_lowering=False, debug=False, num_swdge_queues=4)

    io = {
        "zp4": nc.declare_dram_parameter("zp4", [128, N // 4], F32, isOutput=False),
        "gate": nc.declare_dram_parameter("gate", [N, K], F32, isOutput=False),
        "betacm": nc.declare_dram_parameter("betacm", [128, 128], F32, isOutput=False),
        "degcm": nc.declare_dram_parameter("degcm", [128, 128], F32, isOutput=False),
        "at": nc.declare_dram_parameter("at", [K, D], F32, isOutput=False),
        "sidx": nc.declare_dram_parameter("sidx", [128, S // 16], I16, isOutput=False),
        "sidxd": nc.declare_dram_parameter("sidxd", [128, SROW // 16], I16, isOutput=False),
        "bsf": nc.declare_dram_parameter("bsf", [1, S], F32, isOutput=False),
        "bsdev": nc.declare_dram_parameter("bsdev", [128, SRCH], F32, isOutput=False),
        "ei": nc.declare_dram_parameter("ei", [128, EPAD // 16], I16, isOutput=False),
        "ej": nc.declare_dram_parameter("ej", [128, EPAD // 16], I16, isOutput=False),
        "out": nc.declare_dram_parameter("out", [128, 4], F32, isOutput=True),
    }

    with tile.TileContext(nc) as tc, ExitStack() as ctx:
        _emit(nc, tc, ctx, io)

    nc.compile()
    return nc


_NC_CACHE = None
LAST_RESULTS = None


def _get_program():
    global _NC_CACHE
    if _NC_CACHE is None:
        _NC_CACHE = _build_program()
    return _NC_CACHE


def _wrap_idx(idx, pad_to):
    """Wrap indices into the [128, pad_to//16] int16 layout dma_gather expects."""
    n = idx.shape[0]
    padded = np.full(pad_to, PAD_IDX, dtype=np.int64)
    padded[:n] = idx
    w = padded.reshape(pad_to // 16, 16).T.astype(np.int16)  # [16, L]
    return np.ascontiguousarray(np.tile(w, (8, 1)))  # [128, L]


def kernel(beta, A, Zp, Gate, sample_idx, sparse_i, sparse_j):
    beta = np.ascontiguousarray(np.asarray(beta, dtype=np.float32))
    A = np.ascontiguousarray(np.asarray(A, dtype=np.float32))
    Zp = np.ascontiguousarray(np.asarray(Zp, dtype=np.float32))
    Gate = np.ascontiguousarray(np.asarray(Gate, dtype=np.float32))
    si = np.asarray(sample_idx).astype(np.int64)
    ei = np.asarray(sparse_i).astype(np.int64)
    ej = np.asarray(sparse_j).astype(np.int64)
    # Sort edges by (i, j): the edge sum is order-invariant and sorted indices
    # give the 256B row gathers better HBM locality.
    order = np.lexsort((ej, ei))
    ei = ei[order]
    ej = ej[order]

    nc = _get_program()

    # restack Zp [K, N] -> [128, N/4]: zp4[b*32+k, g*128+p] = Zp[k, (4g+b)*128+p]
    zp4 = np.ascontiguousarray(
        Zp.reshape(K, NTILE, 4, 128).transpose(2, 0, 1, 3).reshape(128, N // 4)
    )
    beta_cm = np.ascontiguousarray(beta.reshape(128, 128, order="F"))
    deg = (np.bincount(ei, minlength=N) + np.bincount(ej, minlength=N)).astype(np.float32)
    deg_cm = np.ascontiguousarray(deg.reshape(128, 128, order="F"))
    at = np.ascontiguousarray(A.T)
    bs = beta[si]
    bsf = np.ascontiguousarray(bs.reshape(1, S))
    sidx_w = _wrap_idx(si, S)

    in_maps = []
    for d in range(NCORE):
        rows = si[d * SROW : (d + 1) * SROW]
        bsdev = np.ascontiguousarray(bs[d * SROW : (d + 1) * SROW].reshape(SRCH, 128).T)
        in_maps.append(
            dict(
                zp4=zp4,
                gate=Gate,
                betacm=beta_cm,
                degcm=deg_cm if d == 0 else np.zeros((128, 128), np.float32),
                at=at,
                sidx=sidx_w,
                sidxd=_wrap_idx(rows, SROW),
                bsf=bsf,
                bsdev=bsdev,
                ei=_wrap_idx(ei[d * EDEV : (d + 1) * EDEV], EPAD),
                ej=_wrap_idx(ej[d * EDEV : (d + 1) * EDEV], EPAD),
            )
        )

    trace = os.environ.get("BASS_KERNEL_TRACE") == "1"
    res = run_bass_kernel_spmd(nc, in_maps, list(range(NCORE)), trace=trace)
    global LAST_RESULTS
    LAST_RESULTS = res

    z2 = 0.0
    z1 = 0.0
    for d in range(NCORE):
        o = np.asarray(res.results[d]["out"], dtype=np.float64)
        z2 += o[:, 0].sum()
        z1 += o[:, 1 : 1 + SRCH].sum()
    e = np.float64(np.exp(np.float32(1.0)))
    return np.float32(z2 - 0.5 * e * e * z1)
